# revision 4
# baseline (speedup 1.0000x reference)
"""Trainium2 Bass kernel for nn_LSTMHarmonizer — V2 (latency-optimized scan).

Data-parallel over batch: 8 cores x 8 sequences. Per core:
  Phase 1: gx = W_ih' @ x + b  (PE GEMM, bf16)
  Phase 2: 1024-step LSTM scan, restructured for minimal critical path:
    - gate tile order (m): f0 g0 i0 o0 f1 g1 i1 o1  (j = h-chunk)
    - train: ident(gx), c-gates k0 (6), c-gates k1 (6), o-gates (4)
      so the c-chains launch ~575ns into the train instead of at the end.
    - per-j chains: sigma(f,g,i) [1 ACT op] -> v,u,c' [3 DVE ops, c' in
      PSUM] -> sc=sigma(c') [ACT, psum read] -> h/2=(sc-.5)*sigma(o) [DVE]
    - c' tracks 2c; g rows pre-scaled 2x (tanh via 2*sigma(2x)-1); h
      stored as h/2 in bf16 (W_hh and head_w pre-scaled 2x).
  Phase 3: 3-head GEMM from the h history + bias add + DMA out.
"""

import contextlib
import numpy as np
import ml_dtypes

import concourse.bass as bass
import concourse.mybir as mybir
from concourse.bass_utils import run_bass_kernel_spmd

BF16 = ml_dtypes.bfloat16
FP8 = ml_dtypes.float8_e4m3fn

B, T, D, H, V, NV = 64, 1024, 128, 256, 128, 3
G4 = 4 * H            # 1024
NC = 8                # cores
BC = B // NC          # 8 sequences per core
NVV = NV * V          # 384

_cache = {}

# k1-phase / o-phase tile orders (m indices)
CJ = [0, 1, 2, 3, 4, 5]       # c-gates: f0 g0 i0, f1 g1 i1
OG = [(6, 0), (7, 0), (6, 1), (7, 1)]  # o-gates (m, k)


def build_nc(Tc=T, w8=False):
    TOK = BC * Tc
    NT3 = TOK // 128          # phase-3 token tiles
    NCH = TOK // 512          # phase-1 token chunks
    f32 = mybir.dt.float32
    bf16 = mybir.dt.bfloat16
    wdt = mybir.dt.float8e4 if w8 else bf16

    nc = bass.Bass()
    xT_d = nc.declare_dram_parameter("xT", [128, TOK], bf16, isOutput=False)
    wih_d = nc.declare_dram_parameter("wihT", [128, G4], bf16, isOutput=False)
    whh_d = nc.declare_dram_parameter("whhT", [128, 16 * 128], wdt, isOutput=False)
    hw_d = nc.declare_dram_parameter("headwT", [128, 2 * NVV], bf16, isOutput=False)
    bias_d = nc.declare_dram_parameter("biasm", [128, 8], f32, isOutput=False)
    hb_d = nc.declare_dram_parameter("headb", [128, NVV], f32, isOutput=False)
    id_d = nc.declare_dram_parameter("ident", [128, 128], bf16, isOutput=False)
    z_d = nc.declare_dram_parameter("zeros16", [128, 16], f32, isOutput=False)
    cst_d = nc.declare_dram_parameter("consts", [128, 2], f32, isOutput=False)
    lg_d = nc.declare_dram_parameter("logits", [NV, BC, Tc, V], f32, isOutput=True)

    ctx = contextlib.ExitStack()
    with ctx:
        sb = lambda name, shape, dt: ctx.enter_context(
            nc.sbuf_tensor(name, shape, dt))
        ps = lambda name, shape: ctx.enter_context(
            nc.psum_tensor(name, shape, f32))
        sem = lambda name: ctx.enter_context(nc.semaphore(name))

        xT = sb("xT_s", [128, TOK], bf16)
        wih = sb("wih_s", [128, G4], bf16)
        whh = sb("whh_s", [128, 16 * 128], wdt)
        hw = sb("hw_s", [128, 2 * NVV], bf16)
        biasm = sb("biasm_s", [128, 8], f32)
        headb = sb("headb_s", [128, NVV], f32)
        ident = sb("ident_s", [128, 128], bf16)
        zeros16 = sb("zeros16_s", [128, 16], f32)
        cst = sb("cst_s", [128, 2], f32)   # col0=0.5 col1=4.0
        gx = sb("gx_s", [128, 8 * TOK], bf16)      # (t, m, b)
        hh = sb("hh_s", [128, 2 * TOK], bf16)      # (j, t, b), holds h/2
        sall = sb("sall_s", [128, 64], f32)        # sigma of gates (m, b)
        scb = sb("scb_s", [128, 16], f32)          # sigma(c') per (j, b)
        vu = sb("vu_s", [128, 32], f32)            # v (0:16), u (16:32)
        cps = sb("cps_s", [128, 16], f32)          # c' (=2c): j0 0:8, j1 8:16
        outb = sb("outb_s", [128, 4 * NVV], f32)   # phase-3 evict slots

        # 6 full banks: [0..3] phases 1/3; scan uses [t%2], [2+t%2], [4+t%2]
        ps_big = [ps(f"psb{i}", [128, 512]) for i in range(6)]

        dma_in = sem("dma_in")
        mm1 = sem("mm1")
        ev1a = sem("ev1a")
        ev1d = sem("ev1d")
        s_mm = sem("s_mm")     # 3/step: c-j0, c-j1, o
        s_sig = sem("s_sig")   # 5/step: A1 A3 A2' sc0 sc1
        s_cp = sem("s_cp")     # init=2, then 2/step: c'0 (2t+3), c'1 (2t+4)
        s_vu = sem("s_vu")     # 2/step: u0 (2t+1), u1 (2t+2)
        s_h = sem("s_h")       # 2/step: h0, h1
        mm3 = sem("mm3")
        ev3 = sem("ev3")
        dma_out = sem("dma_out")

        ALU = mybir.AluOpType
        AF = mybir.ActivationFunctionType

        gx_v = gx[:].rearrange("p (t m b) -> p t m b", t=Tc, m=8, b=BC)

        def gx_evict_view(m, ch):
            return gx_v[:, ch * 64:(ch + 1) * 64, m, :]

        def gx_step_ap(t):
            return gx[:, t * 64:(t + 1) * 64]

        def hh_ap(j, t):
            off = j * TOK + t * BC
            return hh[:, off:off + BC]

        def bc0(t):
            return ps_big[t % 2]
        def bc1(t):
            return ps_big[2 + t % 2]
        def bo(t):
            return ps_big[4 + t % 2]

        def wt(k, m):
            return whh[:, (k * 8 + m) * 128:(k * 8 + m + 1) * 128]

        with nc.Block() as block:

            @block.sync
            def _(sync):
                sync.dma_start(out=xT[:], in_=xT_d[:]).then_inc(dma_in, 16)
                sync.dma_start(out=wih[:], in_=wih_d[:]).then_inc(dma_in, 16)
                sync.dma_start(out=whh[:], in_=whh_d[:]).then_inc(dma_in, 16)
                sync.dma_start(out=hw[:], in_=hw_d[:]).then_inc(dma_in, 16)
                sync.dma_start(out=biasm[:], in_=bias_d[:]).then_inc(dma_in, 16)
                sync.dma_start(out=headb[:], in_=hb_d[:]).then_inc(dma_in, 16)
                sync.dma_start(out=ident[:], in_=id_d[:]).then_inc(dma_in, 16)
                sync.dma_start(out=zeros16[:], in_=z_d[:]).then_inc(dma_in, 16)
                sync.dma_start(out=cst[:], in_=cst_d[:]).then_inc(dma_in, 16)
                # phase 3 output DMAs
                for tk in range(NT3):
                    sync.wait_ge(ev3, tk + 1)
                    for n in range(NV):
                        dview = lg_d[n, :, tk * 16:(tk + 1) * 16, :].rearrange(
                            "b t v -> t b v")
                        slot = outb[:, (tk % 4) * NVV + n * V:
                                    (tk % 4) * NVV + (n + 1) * V]
                        sync.dma_start(out=dview, in_=slot).then_inc(dma_out, 16)
                sync.wait_ge(dma_out, 48 * NT3)

            @block.tensor
            def _(tensor):
                tensor.wait_ge(dma_in, 144)
                # ---- phase 1: gx GEMM ----
                for m in range(8):
                    for ch in range(NCH):
                        idx = m * NCH + ch
                        if idx >= 4:
                            j = idx - 4
                            if j % 2 == 0:
                                tensor.wait_ge(ev1a, j // 2 + 1)
                            else:
                                tensor.wait_ge(ev1d, (j + 1) // 2)
                        tensor.matmul(
                            ps_big[idx % 4][:, :512],
                            lhsT=wih[:, m * 128:(m + 1) * 128],
                            rhs=xT[:, ch * 512:(ch + 1) * 512],
                            start=True, stop=True,
                        ).then_inc(mm1, 1)
                # ---- phase 2: scan ----
                tensor.wait_ge(ev1a, 4 * NCH)
                tensor.wait_ge(ev1d, 4 * NCH)
                def gout(t, m):
                    # psum slot for gate tile m of step t
                    if m < 3:
                        return bc0(t)[:, m * 8:(m + 1) * 8]
                    if m < 6:
                        return bc1(t)[:, (m - 3) * 8:(m - 2) * 8]
                    return bo(t)[:, (m - 6) * 8:(m - 5) * 8]

                gxs = lambda t, a, b2: gx[:, t * 64 + a:t * 64 + b2]
                for t in range(Tc):
                    if t >= 2:
                        tensor.wait_ge(s_sig, 5 * (t - 2) + 3)
                    tensor.matmul(
                        bc0(t)[:, 0:24], lhsT=ident[:], rhs=gxs(t, 0, 24),
                        start=True, stop=(t == 0), skip_group_check=True)
                    tensor.matmul(
                        bc1(t)[:, 0:24], lhsT=ident[:], rhs=gxs(t, 24, 48),
                        start=True, stop=(t == 0), skip_group_check=True)
                    ins0 = tensor.matmul(
                        bo(t)[:, 0:16], lhsT=ident[:], rhs=gxs(t, 48, 64),
                        start=True, stop=(t == 0), skip_group_check=True)
                    if t == 0:
                        ins0.then_inc(s_mm, 1)
                        continue
                    # c-gates k0 phase
                    tensor.wait_ge(s_h, 2 * t - 1)
                    for m in CJ:
                        tensor.matmul(
                            gout(t, m), lhsT=wt(0, m),
                            rhs=hh_ap(0, t - 1), start=False, stop=False,
                            skip_group_check=True)
                    # c-gates k1 phase
                    tensor.wait_ge(s_h, 2 * t)
                    for i, m in enumerate(CJ):
                        ins = tensor.matmul(
                            gout(t, m), lhsT=wt(1, m),
                            rhs=hh_ap(1, t - 1), start=False, stop=(i in (2, 5)),
                            skip_group_check=True)
                        if i == 2:
                            ins.then_inc(s_mm, 1)     # c-j0 done: 3t-1
                        elif i == 5:
                            ins.then_inc(s_mm, 1)     # c-j1 done: 3t
                    # o-gates
                    for i, (m, k) in enumerate(OG):
                        ins = tensor.matmul(
                            gout(t, m), lhsT=wt(k, m),
                            rhs=hh_ap(k, t - 1), start=False,
                            stop=(i == 3), skip_group_check=True)
                        if i == 3:
                            ins.then_inc(s_mm, 1)     # o done: 3t+1
                # ---- phase 3: heads ----
                tensor.wait_ge(s_h, 2 * Tc)
                for tk in range(NT3):
                    if tk >= 4:
                        tensor.wait_ge(ev3, tk - 3)
                    tensor.matmul(
                        ps_big[tk % 4][:, :NVV],
                        lhsT=hh[:, tk * 128:tk * 128 + 128],
                        rhs=hw[:, :NVV], start=True, stop=False,
                        skip_group_check=True,
                    )
                    tensor.matmul(
                        ps_big[tk % 4][:, :NVV],
                        lhsT=hh[:, TOK + tk * 128:TOK + tk * 128 + 128],
                        rhs=hw[:, NVV:2 * NVV], start=False, stop=True,
                        skip_group_check=True,
                    ).then_inc(mm3, 1)

            @block.scalar
            def _(scalar):
                scalar.wait_ge(dma_in, 144)
                # phase-1 evicts: even tiles
                for idx in range(0, 8 * NCH, 2):
                    m, ch = idx // NCH, idx % NCH
                    scalar.wait_ge(mm1, idx + 1)
                    scalar.activation(
                        out=gx_evict_view(m, ch),
                        in_=ps_big[idx % 4][:, :512].rearrange(
                            "p (t b) -> p t b", t=64, b=BC),
                        func=AF.Identity, bias=biasm[:, m:m + 1],
                    ).then_inc(ev1a, 1)
                # scan
                for t in range(Tc):
                    scalar.wait_ge(s_mm, 3 * t - 1 if t else 1)
                    scalar.activation(out=sall[:, 0:24], in_=bc0(t)[:, 0:24],
                                      func=AF.Sigmoid).then_inc(s_sig, 1)
                    scalar.wait_ge(s_mm, 3 * t if t else 1)
                    scalar.activation(out=sall[:, 24:48], in_=bc1(t)[:, 0:24],
                                      func=AF.Sigmoid).then_inc(s_sig, 1)
                    scalar.wait_ge(s_mm, 3 * t + 1)
                    scalar.activation(out=sall[:, 48:64], in_=bo(t)[:, 0:16],
                                      func=AF.Sigmoid).then_inc(s_sig, 1)
                    scalar.wait_ge(s_cp, 2 * t + 2)
                    scalar.activation(out=scb[:, 0:8], in_=cps[:, 0:8],
                                      func=AF.Sigmoid).then_inc(s_sig, 1)
                    scalar.wait_ge(s_cp, 2 * t + 3)
                    scalar.activation(out=scb[:, 8:16], in_=cps[:, 8:16],
                                      func=AF.Sigmoid).then_inc(s_sig, 1)

            @block.vector
            def _(vector):
                vector.wait_ge(dma_in, 144)
                # phase-1 evicts: odd tiles
                for idx in range(1, 8 * NCH, 2):
                    m, ch = idx // NCH, idx % NCH
                    vector.wait_ge(mm1, idx + 1)
                    vector.tensor_scalar_add(
                        gx_evict_view(m, ch),
                        ps_big[idx % 4][:, :512].rearrange(
                            "p (t b) -> p t b", t=64, b=BC),
                        biasm[:, m:m + 1],
                    ).then_inc(ev1d, 1)
                # init c' = 0
                vector.tensor_copy(cps[:], zeros16[:]).then_inc(s_cp, 1)
                # scan: sall cols (m,b): f0 g0 i0 o0 f1 g1 i1 o1
                for t in range(Tc):
                    for j in range(2):
                        o = 24 * j
                        vector.wait_ge(s_sig, 5 * t + 1 + j)
                        vector.wait_ge(s_cp, max(1, 2 * t + j))
                        # v_j = sig(f_j) * c'_old_j
                        vector.tensor_tensor(
                            out=vu[:, j * 8:j * 8 + 8], in0=sall[:, o:o + 8],
                            in1=cps[:, j * 8:(j + 1) * 8], op=ALU.mult)
                        # u_j = (sig(2g_j) - 0.5) * sig(i_j)
                        vector.scalar_tensor_tensor(
                            out=vu[:, 16 + j * 8:24 + j * 8],
                            in0=sall[:, o + 8:o + 16], scalar=cst[:, 0:1],
                            in1=sall[:, o + 16:o + 24],
                            op0=ALU.subtract, op1=ALU.mult).then_inc(s_vu, 1)
                    for j in range(2):
                        # c'_j = 4*u_j + v_j
                        vector.wait_ge(s_vu, 2 * t + 1 + j)
                        vector.scalar_tensor_tensor(
                            out=cps[:, j * 8:(j + 1) * 8],
                            in0=vu[:, 16 + j * 8:24 + j * 8], scalar=cst[:, 1:2],
                            in1=vu[:, j * 8:j * 8 + 8],
                            op0=ALU.mult, op1=ALU.add).then_inc(s_cp, 1)
                    for j in range(2):
                        # h_j/2 = (sc_j - 0.5) * sig(o_j)
                        vector.wait_ge(s_sig, 5 * t + 4 + j)
                        vector.scalar_tensor_tensor(
                            out=hh_ap(j, t), in0=scb[:, j * 8:(j + 1) * 8],
                            scalar=cst[:, 0:1],
                            in1=sall[:, 48 + 8 * j:56 + 8 * j],
                            op0=ALU.subtract, op1=ALU.mult).then_inc(s_h, 1)
                # phase-3 evicts
                for tk in range(NT3):
                    vector.wait_ge(mm3, tk + 1)
                    if tk >= 4:
                        vector.wait_ge(dma_out, 48 * (tk - 3))
                    slot = outb[:, (tk % 4) * NVV:(tk % 4 + 1) * NVV]
                    vector.tensor_tensor(
                        out=slot, in0=ps_big[tk % 4][:, :NVV], in1=headb[:],
                        op=ALU.add).then_inc(ev3, 1)

    return nc


def _prep_weights(W_ih, W_hh, b_ih, b_hh, head_w, head_b, w8=False):
    # gate order (i,f,g,o) -> m-tiles (f0 g0 i0 o0 f1 g1 i1 o1)
    a = np.arange
    perm = np.concatenate([
        a(256, 384), a(512, 640), a(0, 128),
        a(384, 512), a(640, 768), a(128, 256),
        a(768, 896), a(896, 1024)])
    g_rows = np.concatenate([a(128, 256), a(512, 640)])  # g0, g1 in new order
    wi = W_ih[perm].astype(np.float64).copy()
    wh = W_hh[perm].astype(np.float64).copy()
    bb = (b_ih + b_hh)[perm].astype(np.float64).copy()
    # tanh fold: g rows x2 everywhere; h stored as h/2: W_hh x2, head_w x2
    wi[g_rows] *= 2.0
    bb[g_rows] *= 2.0
    wh *= 2.0
    wh[g_rows] *= 2.0
    hwn = 2.0 * head_w.astype(np.float64)

    wihT = wi.T.astype(BF16)                       # [D, G4]
    whhT_f = wh.T                                  # [H, G4]
    whh_tiles = np.zeros((128, 16 * 128), np.float64)
    for k in range(2):
        for m in range(8):
            whh_tiles[:, (k * 8 + m) * 128:(k * 8 + m + 1) * 128] = \
                whhT_f[k * 128:(k + 1) * 128, m * 128:(m + 1) * 128]
    hwT = hwn.reshape(NVV, H).T                    # [H, NVV]
    hw_tiles = np.concatenate([hwT[:128], hwT[128:]], axis=1)  # [128, 2*NVV]
    biasm = bb.reshape(8, 128).T.astype(np.float32).copy()     # [128, 8]
    headb = np.broadcast_to(head_b.reshape(NVV)[None, :],
                            (128, NVV)).astype(np.float32).copy()
    ident = np.eye(128, dtype=BF16)
    wdt = FP8 if w8 else BF16
    return (np.ascontiguousarray(wihT),
            np.ascontiguousarray(whh_tiles.astype(wdt)),
            np.ascontiguousarray(hw_tiles.astype(BF16)),
            biasm, headb, ident)


def run(inputs, Tc=T, w8=False, trace=False, ncores=NC):
    x = np.asarray(inputs["x"])[:, :Tc]
    wihT, whh_tiles, hw_tiles, biasm, headb, ident = _prep_weights(
        np.asarray(inputs["W_ih"]), np.asarray(inputs["W_hh"]),
        np.asarray(inputs["b_ih"]), np.asarray(inputs["b_hh"]),
        np.asarray(inputs["head_w"]), np.asarray(inputs["head_b"]), w8=w8)

    key = (Tc, w8)
    if key not in _cache:
        _cache[key] = build_nc(Tc, w8=w8)
    nc = _cache[key]

    consts = np.broadcast_to(np.array([0.5, 4.0], np.float32)[None, :],
                             (128, 2)).copy()
    in_maps = []
    for c in range(ncores):
        xs = x[c * BC:(c + 1) * BC]                    # [BC, Tc, D]
        xTc = np.ascontiguousarray(
            xs.transpose(2, 1, 0).reshape(128, BC * Tc)).astype(BF16)
        in_maps.append({
            "xT": xTc, "wihT": wihT, "whhT": whh_tiles, "headwT": hw_tiles,
            "biasm": biasm, "headb": headb, "ident": ident,
            "zeros16": np.zeros((128, 16), np.float32),
            "consts": consts,
        })

    res = run_bass_kernel_spmd(nc, in_maps, core_ids=list(range(ncores)),
                               trace=trace)
    outs = [r["logits"] for r in res.results]          # [NV, BC, Tc, V] each
    full = np.concatenate(outs, axis=1)                # [NV, B', Tc, V]
    return (full[0], full[1], full[2]), res


def kernel(x, W_ih, W_hh, b_ih, b_hh, head_w, head_b):
    outs, _ = run(dict(x=x, W_ih=W_ih, W_hh=W_hh, b_ih=b_ih, b_hh=b_hh,
                       head_w=head_w, head_b=head_b))
    return outs
